# revision 8
# baseline (speedup 1.0000x reference)
"""CosineSimAttention Trainium2 kernel (8 NeuronCores, SPMD).

Problem: B=2, S=2048, D=1024, H=16 heads of DH=64.
    q = l2norm(x@w_q) * exp(scale);  k = l2norm(x@w_kv[:, :INNER])
    out = softmax(q k^T) v @ w_out + b_out

Sharding: heads are split across the 8 cores (2 heads/core); every core sees
the full (host-pre-transposed) activation x^T and its own column/row slices of
the projection weights.  Each core computes a row-parallel partial of the
output projection; the host sums the 8 partials (the unshard step).

Per-core dataflow (all matmuls bf16 operands, fp32 PSUM accumulation):
  xT[d,tok] -> qT/kT [128(2h x dh), 4096] and v [tok, dh] projections
  q-norm:  ssq via ones-matmul -> ACT sqrt -> DVE recip -> *exp(scale)
           -> PE broadcast to [128, tok] -> DVE mul -> qhatT (bf16)
  k-norm:  folded into the softmax exp as a per-partition ACT scale (rows of
           simT are keys, so 1/||k_j|| is a [128,1] activation scale).
  attn:    simT[j,i] = kT.T qT per 128-key chunk, exp on ACT, AV accumulates
           outT[65, i] with a ones-column in v_aug producing the softmax
           denominator row; normalize with reciprocal + PE broadcast.
  out-proj: y[tok,:] = attnT.T @ w_out_slice  (partial; host reduces).
"""

import math

import numpy as np

import concourse.bass as bass
import concourse.mybir as mybir
import concourse.tile as tile
from concourse import bacc, bass_utils

B, S, D, H, DH = 2, 2048, 1024, 16, 64
INNER = H * DH
T = B * S                    # 4096 tokens
NCORES = 8
HPC = H // NCORES            # heads per core = 2
HD = HPC * DH                # 128 projection cols per core

FP32 = mybir.dt.float32
BF16 = mybir.dt.bfloat16
AF = mybir.ActivationFunctionType

NCHUNK = T // 512            # 8 token chunks of 512
MCHUNK = T // 128            # 32 token chunks of 128
JCH = S // 128               # 16 key chunks per batch
IBLK = 1024                  # query extent per attention block
VA_W = 2 * (DH + 1)          # v_aug columns: [vA | 1 | vB | 1]


def build(nc: bacc.Bacc, debug_dumps=False):
    xT = nc.dram_tensor("xT", [D, T], FP32, kind="ExternalInput").ap()
    wq = nc.dram_tensor("wq", [D, HD], FP32, kind="ExternalInput").ap()
    wk = nc.dram_tensor("wk", [D, HD], FP32, kind="ExternalInput").ap()
    wv = nc.dram_tensor("wv", [D, HD], FP32, kind="ExternalInput").ap()
    wo = nc.dram_tensor("wo", [HD, D], FP32, kind="ExternalInput").ap()
    sc = nc.dram_tensor("sc", [HPC, 1], FP32, kind="ExternalInput").ap()
    onesr_d = nc.dram_tensor("onesr", [HPC, 128], BF16, kind="ExternalInput").ap()
    onesc_d = nc.dram_tensor("onesc", [128, HPC], BF16, kind="ExternalInput").ap()
    ones64_d = nc.dram_tensor("ones64", [1, DH], BF16, kind="ExternalInput").ap()
    out = nc.dram_tensor("out", [T, D], FP32, kind="ExternalOutput").ap()
    dbg = {}
    if debug_dumps:
        for nm, shp, dt in (
            ("d_qT", [128, T], BF16),
            ("d_kT", [128, T], BF16),
            ("d_attnT", [128, T], BF16),
            ("d_va", [128, MCHUNK * VA_W], BF16),
            ("d_rnkT", [128, 2 * MCHUNK], FP32),
            ("d_pe", [128, IBLK], BF16),
            ("d_oacc", [DH + 1, IBLK], FP32),
            ("d_rc", [1, IBLK], FP32),
            ("d_bca", [DH, IBLK], FP32),
            ("d_attf", [DH, IBLK], FP32),
        ):
            dbg[nm] = nc.dram_tensor(nm, shp, dt, kind="ExternalOutput").ap()

    with tile.TileContext(nc) as tc:
        from contextlib import ExitStack

        with ExitStack() as ctx:
            ep = ctx.enter_context  # noqa

            consts = ep(tc.tile_pool(name="consts", bufs=1))
            big = ep(tc.tile_pool(name="big", bufs=1))
            sbw = ep(tc.tile_pool(name="sbw", bufs=2))
            sbx = ep(tc.tile_pool(name="sbx", bufs=2))
            sbp = ep(tc.tile_pool(name="sbp", bufs=2))
            sbe = ep(tc.tile_pool(name="sbe", bufs=3))
            sba = ep(tc.tile_pool(name="sba", bufs=2))
            sby = ep(tc.tile_pool(name="sby", bufs=2))

            # ---- constants (host-shipped block-ones patterns) --------------
            onesr = consts.tile([HPC, 128], BF16, tag="onesr")
            nc.sync.dma_start(onesr[:], onesr_d[:, :])
            onesc = consts.tile([128, HPC], BF16, tag="onesc")
            nc.sync.dma_start(onesc[:], onesc_d[:, :])
            ones64 = consts.tile([1, DH], BF16, tag="ones64")
            nc.sync.dma_start(ones64[:], ones64_d[:, :])

            # t = exp(scale) per head, as a [HPC,1] per-partition scalar
            sc_sb = consts.tile([HPC, 1], FP32, tag="sc_sb")
            nc.sync.dma_start(sc_sb[:], sc[:, :])
            t_sb = consts.tile([HPC, 1], FP32, tag="t_sb")
            nc.scalar.activation(t_sb[:], sc_sb[:], AF.Exp)

            # ---- weights ---------------------------------------------------
            wq_b = consts.tile([128, D], BF16, tag="wq_b")
            wk_b = consts.tile([128, D], BF16, tag="wk_b")
            wv_b = consts.tile([128, D], BF16, tag="wv_b")
            wo_b = consts.tile([128, D], BF16, tag="wo_b")
            for name, dram, bt in (("q", wq, wq_b), ("k", wk, wk_b), ("v", wv, wv_b)):
                wf = sbw.tile([128, D], FP32, tag="wf")
                for p in range(8):
                    nc.sync.dma_start(
                        wf[:, p * 128 : (p + 1) * 128],
                        dram[p * 128 : (p + 1) * 128, :],
                    )
                nc.vector.tensor_copy(bt[:], wf[:])
            wof = sbw.tile([128, D], FP32, tag="wf")
            nc.sync.dma_start(wof[:], wo[:, :])
            nc.vector.tensor_copy(wo_b[:], wof[:])

            # ---- x^T load + bf16 convert ----------------------------------
            xT_b = big.tile([128, 8 * T], BF16, tag="xT_b")  # [d128-chunk p] at cols p*T
            for p in range(8):
                for half in range(2):
                    xf = sbx.tile([128, T // 2], FP32, tag="xf")
                    nc.sync.dma_start(
                        xf[:],
                        xT[p * 128 : (p + 1) * 128, half * (T // 2) : (half + 1) * (T // 2)],
                    )
                    nc.vector.tensor_copy(
                        xT_b[:, p * T + half * (T // 2) : p * T + (half + 1) * (T // 2)],
                        xf[:],
                    )

            # ---- persistent activations -----------------------------------
            qT_b = big.tile([128, T], BF16, tag="qT_b")
            kT_b = big.tile([128, T], BF16, tag="kT_b")
            attnT = big.tile([128, T], BF16, tag="attnT")
            va = big.tile([128, MCHUNK * VA_W], BF16, tag="va")
            nc.vector.memset(va[:], 1.0)  # ones columns survive the copies below
            rnkT = big.tile([128, 2 * MCHUNK], FP32, tag="rnkT")

            with tc.tile_pool(name="pmm", bufs=3, space="PSUM") as pmm, \
                 tc.tile_pool(name="psml", bufs=2, space="PSUM") as psml:
                # ---- q / k projections + norms, per 512-token chunk -------
                for n in range(NCHUNK):
                    t0 = n * 512
                    qp = pmm.tile([128, 512], FP32, tag="mm")
                    for p in range(8):
                        nc.tensor.matmul(
                            qp[:],
                            wq_b[:, p * 128 : (p + 1) * 128],
                            xT_b[:, p * T + t0 : p * T + t0 + 512],
                            start=(p == 0),
                            stop=(p == 7),
                        )
                    # sumsq per head -> [2, 512]
                    sqq = sbp.tile([128, 512], BF16, tag="sqq")
                    nc.scalar.activation(sqq[:], qp[:], AF.Square)
                    ssq = psml.tile([HPC, 512], FP32, tag="sml")
                    nc.tensor.matmul(ssq[:], onesc[:], sqq[:], start=True, stop=True)
                    nrm = sbp.tile([HPC, 512], FP32, tag="nrm")
                    nc.scalar.activation(nrm[:], ssq[:], AF.Sqrt)
                    rnq = sbp.tile([HPC, 512], FP32, tag="rnq")
                    nc.vector.reciprocal_approx_fast(rnq[:], nrm[:])
                    rnqt = sbp.tile([HPC, 512], BF16, tag="rnqt")
                    nc.vector.tensor_scalar_mul(rnqt[:], rnq[:], t_sb[:])
                    # broadcast [2,512] -> [128,512] and apply
                    bc = pmm.tile([128, 512], FP32, tag="mm")
                    nc.tensor.matmul(bc[:], onesr[:], rnqt[:], start=True, stop=True)
                    bcs = sbp.tile([128, 512], FP32, tag="bcs")
                    nc.scalar.activation(bcs[:], bc[:], AF.Copy)
                    nc.vector.tensor_mul(qT_b[:, t0 : t0 + 512], qp[:], bcs[:])

                    kp = pmm.tile([128, 512], FP32, tag="mm")
                    for p in range(8):
                        nc.tensor.matmul(
                            kp[:],
                            wk_b[:, p * 128 : (p + 1) * 128],
                            xT_b[:, p * T + t0 : p * T + t0 + 512],
                            start=(p == 0),
                            stop=(p == 7),
                        )
                    nc.scalar.activation(kT_b[:, t0 : t0 + 512], kp[:], AF.Copy)
                    sqk = sbp.tile([128, 512], BF16, tag="sqk")
                    nc.scalar.activation(sqk[:], kp[:], AF.Square)
                    # per-token sumsq, transposed: [128 tok, 2]
                    sst = psml.tile([128, 8], FP32, tag="sml")
                    for c4 in range(4):
                        nc.tensor.matmul(
                            sst[:, c4 * HPC : (c4 + 1) * HPC],
                            sqk[:, c4 * 128 : (c4 + 1) * 128],
                            onesc[:],
                            start=True,
                            stop=True,
                        )
                    snt = sbp.tile([128, 8], FP32, tag="snt")
                    nc.scalar.activation(snt[:], sst[:], AF.Sqrt)
                    nc.vector.reciprocal_approx_fast(
                        rnkT[:, n * 8 : n * 8 + 8], snt[:]
                    )

            # ---- v projection (natural layout) ----------------------------
            with tc.tile_pool(name="pv", bufs=2, space="PSUM") as pv:
                for m in range(MCHUNK):
                    vp = pv.tile([128, HD], FP32, tag="v")
                    for p in range(8):
                        nc.tensor.matmul(
                            vp[:],
                            xT_b[:, p * T + m * 128 : p * T + (m + 1) * 128],
                            wv_b[:, p * 128 : (p + 1) * 128],
                            start=(p == 0),
                            stop=(p == 7),
                        )
                    for h in range(HPC):
                        nc.vector.tensor_copy(
                            va[
                                :,
                                m * VA_W + h * (DH + 1) : m * VA_W + h * (DH + 1) + DH,
                            ],
                            vp[:, h * DH : (h + 1) * DH],
                        )

            # ---- attention -------------------------------------------------
            with tc.tile_pool(name="patt", bufs=2, space="PSUM") as patt:
                for b in range(B):
                    for h in range(HPC):
                        hr = slice(h * DH, (h + 1) * DH)
                        for ib in range(S // IBLK):
                            i0 = b * S + ib * IBLK
                            oacc = patt.tile([DH + 1, IBLK], FP32, tag="oacc")
                            for j in range(JCH):
                                c = b * JCH + j
                                st = patt.tile([128, IBLK], FP32, tag="sim")
                                for ih in range(IBLK // 512):
                                    nc.tensor.matmul(
                                        st[:, ih * 512 : (ih + 1) * 512],
                                        kT_b[hr, c * 128 : (c + 1) * 128],
                                        qT_b[hr, i0 + ih * 512 : i0 + (ih + 1) * 512],
                                        start=True,
                                        stop=True,
                                    )
                                pe_t = sbe.tile([128, IBLK], BF16, tag="expT")
                                nc.scalar.activation(
                                    pe_t[:],
                                    st[:],
                                    AF.Exp,
                                    scale=rnkT[:, 2 * c + h : 2 * c + h + 1],
                                )
                                if debug_dumps and b == 0 and h == 0 and ib == 0 and j == 0:
                                    nc.sync.dma_start(dbg["d_pe"][:, :], pe_t[:])
                                for ih in range(IBLK // 512):
                                    nc.tensor.matmul(
                                        oacc[:, ih * 512 : (ih + 1) * 512],
                                        va[
                                            :,
                                            c * VA_W
                                            + h * (DH + 1) : c * VA_W
                                            + (h + 1) * (DH + 1),
                                        ],
                                        pe_t[:, ih * 512 : (ih + 1) * 512],
                                        start=(j == 0),
                                        stop=(j == JCH - 1),
                                    )
                            # normalize: recip of denominator row, broadcast, mul
                            den = sba.tile([1, IBLK], FP32, tag="den")
                            nc.vector.tensor_copy(den[:], oacc[DH : DH + 1, :])
                            rc = sba.tile([1, IBLK], FP32, tag="rc")
                            nc.vector.reciprocal_approx_fast(rc[:], den[:])
                            rcb = sba.tile([1, IBLK], BF16, tag="rcb")
                            nc.vector.tensor_copy(rcb[:], rc[:])
                            bca = patt.tile([128, IBLK], FP32, tag="sim")
                            for ih in range(IBLK // 512):
                                nc.tensor.matmul(
                                    bca[0:DH, ih * 512 : (ih + 1) * 512],
                                    ones64[:],
                                    rcb[:, ih * 512 : (ih + 1) * 512],
                                    start=True,
                                    stop=True,
                                )
                            att_f = sba.tile([DH, IBLK], FP32, tag="attf")
                            nc.scalar.activation(att_f[:], oacc[0:DH, :], AF.Copy)
                            if debug_dumps and b == 0 and h == 0 and ib == 0:
                                do = sba.tile([DH + 1, IBLK], FP32, tag="do")
                                nc.vector.tensor_copy(do[:], oacc[:])
                                nc.sync.dma_start(dbg["d_oacc"][:, :], do[:])
                                nc.sync.dma_start(dbg["d_rc"][:, :], rc[:])
                                db = sba.tile([DH, IBLK], FP32, tag="db")
                                nc.vector.tensor_copy(db[:], bca[0:DH, :])
                                nc.sync.dma_start(dbg["d_bca"][:, :], db[:])
                                nc.sync.dma_start(dbg["d_attf"][:, :], att_f[:])
                            nc.vector.tensor_mul(
                                attnT[hr, i0 : i0 + IBLK], att_f[:], bca[0:DH, :]
                            )

            if debug_dumps:
                nc.sync.dma_start(dbg["d_qT"][:, :], qT_b[:])
                nc.sync.dma_start(dbg["d_kT"][:, :], kT_b[:])
                nc.sync.dma_start(dbg["d_attnT"][:, :], attnT[:])
                nc.sync.dma_start(dbg["d_va"][:, :], va[:])
                nc.sync.dma_start(dbg["d_rnkT"][:, :], rnkT[:])

            # ---- output projection (row-parallel partial) ------------------
            with tc.tile_pool(name="py", bufs=2, space="PSUM") as py:
                for m in range(MCHUNK):
                    yp = py.tile([128, D], FP32, tag="y")
                    for no in range(D // 512):
                        nc.tensor.matmul(
                            yp[:, no * 512 : (no + 1) * 512],
                            attnT[:, m * 128 : (m + 1) * 128],
                            wo_b[:, no * 512 : (no + 1) * 512],
                            start=True,
                            stop=True,
                        )
                    ys = sby.tile([128, D], FP32, tag="ys")
                    nc.scalar.activation(ys[:, 0:512], yp[:, 0:512], AF.Copy)
                    nc.vector.tensor_copy(ys[:, 512:D], yp[:, 512:D])
                    nc.sync.dma_start(out[m * 128 : (m + 1) * 128, :], ys[:])

    nc.compile()
    return nc


_NC = None


def _get_nc():
    global _NC
    if _NC is None:
        _NC = build(
            bacc.Bacc("TRN2", target_bir_lowering=False, debug=False, num_devices=NCORES)
        )
    return _NC


import ml_dtypes

_ONESR = np.zeros((HPC, 128), ml_dtypes.bfloat16)
for _h in range(HPC):
    _ONESR[_h, _h * DH : (_h + 1) * DH] = 1
_ONESC = np.ascontiguousarray(_ONESR.T)
_ONES64 = np.ones((1, DH), ml_dtypes.bfloat16)


def make_in_maps(x, w_q, w_kv, w_out, scale):
    x2 = np.ascontiguousarray(np.asarray(x, np.float32).reshape(T, D).T)
    w_q = np.asarray(w_q, np.float32)
    w_kv = np.asarray(w_kv, np.float32)
    w_out = np.asarray(w_out, np.float32)
    sc = np.asarray(scale, np.float32).reshape(H)
    in_maps = []
    for c in range(NCORES):
        cols = slice(c * HD, (c + 1) * HD)
        in_maps.append(
            {
                "xT": x2,
                "wq": np.ascontiguousarray(w_q[:, cols]),
                "wk": np.ascontiguousarray(w_kv[:, cols]),
                "wv": np.ascontiguousarray(w_kv[:, INNER + c * HD : INNER + (c + 1) * HD]),
                "wo": np.ascontiguousarray(w_out[cols, :]),
                "sc": np.ascontiguousarray(sc[c * HPC : (c + 1) * HPC].reshape(HPC, 1)),
                "onesr": _ONESR,
                "onesc": _ONESC,
                "ones64": _ONES64,
            }
        )
    return in_maps


def kernel(x, w_q, w_kv, w_out, b_out, scale, _trace=False):
    nc = _get_nc()
    in_maps = make_in_maps(x, w_q, w_kv, w_out, scale)
    res = bass_utils.run_bass_kernel_spmd(
        nc, in_maps, core_ids=list(range(NCORES)), trace=_trace
    )
    acc = np.zeros((T, D), np.float64)
    for r in res.results:
        acc += r["out"].astype(np.float64)
    y = (acc + np.asarray(b_out, np.float64)[None, :]).astype(np.float32)
    out = y.reshape(B, S, D)
    if _trace:
        return out, res
    return out
